# revision 4
# baseline (speedup 1.0000x reference)
"""Trainium2 Bass kernel for nn_BCE_Loss (retrieval_knn).

Distributed strategy (8 NeuronCores, SPMD):
  - Each core receives the full batch, ROTATED so that its own 1024 rows come
    first (row-stripe sharding with a replicated right operand; rotation makes
    the SPMD program identical across cores: core c's local row r == global row
    (r + 1024*c) % 8192, likewise columns).
  - On-device per core: L2-normalize rows (f32), cast bf16, transpose via PE
    into xT [512, 8192]; compute the [1024, 8192] cosine stripe tile-by-tile
    through PSUM (bf16 matmul, f32 accumulate); mask the self-match diagonal
    by subtracting 1000 on the (static, thanks to rotation) diagonal blocks;
    per 512-column block take top-8 values + indices (DVE max8/max_index);
    pack (round(v*2048), column) into a single f32 with exact integer
    arithmetic: p = round(v*2048)*8192 + col; merge the 128 candidates per row
    with 3 x (max8 + match_replace) -> sorted top-24 packed values.
  - Host: decode (value, column) from the packed top-24, map columns back to
    global ids, gather labels, and compute the BCE loss (tiny: 8192 x 20).

The per-block top-8 union provably contains the true top-k for k <= 8 per
block; for k=20 over 16 blocks the union miss probability is ~1e-6 per row
and empirically zero effect on the loss (validated vs the reference).
"""

from contextlib import ExitStack

import numpy as np

import concourse.bass as bass
import concourse.mybir as mybir
import concourse.tile as tile
from concourse.bass import ts
from concourse.bass_utils import run_bass_kernel_spmd
from concourse.masks import make_identity
from concourse.vector_clock import ScopedClock, VectorClock

F32 = mybir.dt.float32
BF16 = mybir.dt.bfloat16
U32 = mybir.dt.uint32
I32 = mybir.dt.int32
AF = mybir.ActivationFunctionType
ALU = mybir.AluOpType

B, D = 8192, 512
M = 8              # cores
BL = B // M        # 1024 rows per core
NRT = BL // 128    # 8 row tiles per core
NCB = B // 512     # 16 column blocks
MAGIC = 12582912.0  # 1.5 * 2**23: adding+subtracting rounds to nearest int
NEG = -3.0e38


# ---------------------------------------------------------------------------
# Environment workarounds: this container's walrus accepts at most ONE sem
# wait per instruction, and its runtime crashes on the explicit EventSemaphore
# butterfly barrier TileContext emits at its tail.
# ---------------------------------------------------------------------------

def _patched_drain_and_barrier(self, tick_clock, wait_clock):
    nc = self.nc
    vc = tick_clock.global_clock
    n = len(vc)
    for p in range(n):
        t = vc[p]
        if t > 0:
            pvc = VectorClock([0] * n)
            pvc.require_at_least(p, t)
            nop = nc.sync.nop()
            wait_clock.add_sem_waits(nop.ins, ScopedClock({None: pvc}))
    nc.sync.drain()
    nc._nrt_pseudo_barrier()
    assert self.sems is not None
    popped = nc._tile_sem_poison_stack.pop()
    assert popped is self._sem_poison
    nc.clear_and_free_semaphores(list(self.sems.allocated().values()))
    nc._nrt_pseudo_barrier()


tile.TileContext._drain_and_barrier = _patched_drain_and_barrier


def _split_multi_waits(nc):
    import bass_rust

    for f in nc.m.functions:
        for bb in f.blocks:
            out = []
            changed = False
            for ins in bb.instructions:
                si = ins.sync_info
                waits = list(si.on_wait) if si is not None else []
                if len(waits) > 1:
                    changed = True
                    for w in waits[:-1]:
                        nop = mybir.InstNoOp(
                            name=f"I-wsplit-{nc.next_id()}", ins=[], outs=[]
                        )
                        nop.engine = ins.engine
                        nop.sync_info = bass_rust.SyncInfo(on_wait=[w], on_update=[])
                        out.append(nop)
                    ins.sync_info = bass_rust.SyncInfo(
                        on_wait=[waits[-1]], on_update=list(si.on_update)
                    )
                out.append(ins)
            if changed:
                bb.instructions = out


# ---------------------------------------------------------------------------
# Kernel build
# ---------------------------------------------------------------------------

def build_nc():
    nc = bass.Bass(num_devices=M)
    x = nc.declare_dram_parameter("x", [B, D], F32, isOutput=False)
    out = nc.declare_dram_parameter("out", [BL, 24], F32, isOutput=True)

    with tile.TileContext(nc) as tc, ExitStack() as octx:
        cpool = octx.enter_context(tc.tile_pool(name="const", bufs=1))
        ident_bf = cpool.tile([128, 128], BF16)
        make_identity(nc, ident_bf[:])
        # identity * 1000 for the diagonal (self-similarity) mask
        i1000 = cpool.tile([128, 128], F32)
        nc.gpsimd.memset(i1000[:], 0.0)
        nc.gpsimd.affine_select(
            out=i1000[:], in_=i1000[:], compare_op=ALU.not_equal,
            fill=1000.0, base=0, pattern=[[-1, 128]], channel_multiplier=1,
        )
        # off[p, i] = 512 * (i // 8): column-block base for candidate i
        off_i = cpool.tile([128, 128], I32)
        nc.gpsimd.iota(off_i[:], pattern=[[512, 16], [0, 8]], base=0,
                       channel_multiplier=0)
        off_f = cpool.tile([128, 128], F32)
        nc.scalar.copy(off_f[:], off_i[:])

        xt_pool = octx.enter_context(tc.tile_pool(name="xt", bufs=1))
        xt = [
            xt_pool.tile([128, B], BF16, tag=f"xt{d}", name=f"xt{d}")
            for d in range(4)
        ]

        # Phase 1: normalize rows, cast bf16, transpose into xT
        with tc.tile_pool(name="ld", bufs=3) as ld, \
             tc.tile_pool(name="sm", bufs=4) as sm, \
             tc.tile_pool(name="tp", bufs=4, space="PSUM") as tpp:
            for rt in range(B // 128):
                xtile = ld.tile([128, D], F32, tag="xtile")
                nc.sync.dma_start(xtile[:], x[ts(rt, 128), :])
                sq = ld.tile([128, D], F32, tag="sq")
                ss = sm.tile([128, 1], F32, tag="ss")
                nc.scalar.activation(sq[:], xtile[:], AF.Square,
                                     accum_out=ss[:])
                nrm = sm.tile([128, 1], F32, tag="nrm")
                nc.scalar.sqrt(nrm[:], ss[:])
                rcp = sm.tile([128, 1], F32, tag="rcp")
                nc.vector.reciprocal(rcp[:], nrm[:])
                xbf = ld.tile([128, D], BF16, tag="xbf")
                nc.scalar.activation(xbf[:], xtile[:], AF.Copy, scale=rcp[:])
                for d4 in range(4):
                    tps = tpp.tile([128, 128], BF16, tag="tp")
                    nc.tensor.transpose(tps[:], xbf[:, ts(d4, 128)], ident_bf[:])
                    nc.vector.tensor_copy(xt[d4][:, ts(rt, 128)], tps[:])

        # Phase 2+3: stripe matmul, per-block top-8, pack, merge to top-24
        with tc.tile_pool(name="mm", bufs=8, space="PSUM") as mm, \
             tc.tile_pool(name="cand", bufs=2) as cand, \
             tc.tile_pool(name="fin", bufs=2) as fin:
            for m in range(NRT):
                vals = cand.tile([128, 128], F32, tag="VALS")
                idx = cand.tile([128, 128], U32, tag="IDX")
                for grp in range(2):
                    cbs = list(range(grp * 8, grp * 8 + 8))
                    pss = [
                        mm.tile([128, 512], F32, tag="ps", name=f"ps_{m}_{cb}")
                        for cb in cbs
                    ]
                    for d4 in range(4):
                        lhsT = xt[d4][:, ts(m, 128)]
                        for j, cb in enumerate(cbs):
                            nc.tensor.matmul(
                                pss[j][:], lhsT, xt[d4][:, ts(cb, 512)],
                                start=(d4 == 0), stop=(d4 == 3),
                            )
                    for j, cb in enumerate(cbs):
                        if cb == m // 4:
                            o = (m % 4) * 128
                            nc.vector.tensor_tensor(
                                pss[j][:, o:o + 128], pss[j][:, o:o + 128],
                                i1000[:], op=ALU.subtract,
                            )
                        nc.vector.max(vals[:, cb * 8:cb * 8 + 8], pss[j][:])
                        nc.vector.max_index(
                            idx[:, cb * 8:cb * 8 + 8],
                            vals[:, cb * 8:cb * 8 + 8], pss[j][:],
                        )
                # pack: p = round(v * 2048) * 8192 + (idx + 512 * (i // 8))
                vq = fin.tile([128, 128], F32, tag="vq")
                nc.scalar.activation(vq[:], vals[:], AF.Copy, scale=2048.0,
                                     bias=MAGIC)
                q = fin.tile([128, 128], F32, tag="q")
                nc.vector.tensor_scalar_add(q[:], vq[:], -MAGIC)
                idxf = fin.tile([128, 128], F32, tag="idxf")
                nc.scalar.copy(idxf[:], idx[:])
                t1 = fin.tile([128, 128], F32, tag="t1")
                nc.vector.tensor_tensor(t1[:], idxf[:], off_f[:], op=ALU.add)
                p0 = fin.tile([128, 128], F32, tag="p0")
                nc.vector.scalar_tensor_tensor(
                    p0[:], in0=q[:], scalar=8192.0, in1=t1[:],
                    op0=ALU.mult, op1=ALU.add,
                )
                pv = fin.tile([128, 24], F32, tag="pv")
                p1 = fin.tile([128, 128], F32, tag="p1")
                p2 = fin.tile([128, 128], F32, tag="p2")
                nc.vector.max(pv[:, 0:8], p0[:])
                nc.vector.match_replace(p1[:], pv[:, 0:8], p0[:], NEG)
                nc.vector.max(pv[:, 8:16], p1[:])
                nc.vector.match_replace(p2[:], pv[:, 8:16], p1[:], NEG)
                nc.vector.max(pv[:, 16:24], p2[:])
                nc.sync.dma_start(out[ts(m, 128), :], pv[:])

    _split_multi_waits(nc)
    return nc


_NC = None


def _get_nc():
    global _NC
    if _NC is None:
        _NC = build_nc()
    return _NC


def run_device(x32, trace=False, **kwargs):
    """Run the SPMD kernel; returns (pv [B, 24] f32, BassKernelResults)."""
    nc = _get_nc()
    in_maps = [
        {"x": np.ascontiguousarray(np.roll(x32, -c * BL, axis=0))}
        for c in range(M)
    ]
    res = run_bass_kernel_spmd(nc, in_maps, core_ids=list(range(M)),
                               trace=trace, **kwargs)
    pv = np.concatenate([res.results[c]["out"] for c in range(M)], axis=0)
    return pv, res


def decode_loss(pv, labels, k):
    """Decode packed top-24 -> (values, global column ids) -> BCE loss."""
    pv64 = pv.astype(np.float64)
    q = np.floor(pv64 / 8192.0)
    col = (pv64 - q * 8192.0).astype(np.int64)       # local column in [0, 8192)
    vhat = q / 2048.0                                 # quantized cosine
    vk = vhat[:, :k]
    ck = col[:, :k]
    core = np.arange(B) // BL                         # global row -> core
    gidx = (ck + (core * BL)[:, None]) % B            # local -> global column
    preds = (vk + 1.0) * 0.5
    t = (labels[gidx] == labels[:, None]).astype(np.float64)
    logp = np.maximum(np.log(preds), -100.0)
    log1mp = np.maximum(np.log1p(-preds), -100.0)
    loss = -(t * logp + (1.0 - t) * log1mp)
    return np.float32(loss.mean())


def kernel(batch, labels, k):
    k = int(k)
    assert 0 < k <= 24, f"kernel supports k <= 24, got {k}"
    x32 = np.asarray(batch, dtype=np.float32)
    assert x32.shape == (B, D)
    labels = np.asarray(labels)
    pv, _ = run_device(x32)
    return decode_loss(pv, labels, k)


# revision 12
# speedup vs baseline: 4156.3638x; 4156.3638x over previous
"""Trainium2 Bass kernel for nn_BCE_Loss (retrieval_knn).

Distributed strategy (8 NeuronCores, SPMD):
  - Each core receives the full batch, ROTATED so that its own 1024 rows come
    first (row-stripe sharding with a replicated right operand; rotation makes
    the SPMD program identical across cores: core c's local row r == global row
    (r + 1024*c) % 8192, likewise columns).
  - On-device per core: L2-normalize rows (f32), cast bf16, transpose via PE
    into xT [512, 8192]; compute the [1024, 8192] cosine stripe tile-by-tile
    through PSUM (bf16 matmul, f32 accumulate); mask the self-match diagonal
    by subtracting 1000 on the (static, thanks to rotation) diagonal blocks;
    per 512-column block take top-8 values + indices (DVE max8/max_index);
    pack (round(v*2048), column) into a single f32 with exact integer
    arithmetic: p = round(v*2048)*8192 + col; merge the 128 candidates per row
    with 3 x (max8 + match_replace) -> sorted top-24 packed values.
  - Host: decode (value, column) from the packed top-24, map columns back to
    global ids, gather labels, and compute the BCE loss (tiny: 8192 x 20).

The per-block top-8 union provably contains the true top-k for k <= 8 per
block; for k=20 over 16 blocks the union miss probability is ~1e-6 per row
and empirically zero effect on the loss (validated vs the reference).
"""

from contextlib import ExitStack

import numpy as np

import concourse.bass as bass
import concourse.mybir as mybir
import concourse.tile as tile
from concourse.bass import ts
from concourse.bass_utils import run_bass_kernel_spmd
from concourse.masks import make_identity
from concourse.vector_clock import ScopedClock, VectorClock

F32 = mybir.dt.float32
BF16 = mybir.dt.bfloat16
U32 = mybir.dt.uint32
I32 = mybir.dt.int32
AF = mybir.ActivationFunctionType
ALU = mybir.AluOpType

B, D = 8192, 512
M = 8              # cores
BL = B // M        # 1024 rows per core
NRT = BL // 128    # 8 row tiles per core
NCB = B // 512     # 16 column blocks
MAGIC = 12582912.0  # 1.5 * 2**23: adding+subtracting rounds to nearest int
NEG = -3.0e38


# ---------------------------------------------------------------------------
# Environment workarounds: this container's walrus accepts at most ONE sem
# wait per instruction, and its runtime crashes on the explicit EventSemaphore
# butterfly barrier TileContext emits at its tail.
# ---------------------------------------------------------------------------

def _patched_drain_and_barrier(self, tick_clock, wait_clock):
    nc = self.nc
    vc = tick_clock.global_clock
    n = len(vc)
    for p in range(n):
        t = vc[p]
        if t > 0:
            pvc = VectorClock([0] * n)
            pvc.require_at_least(p, t)
            nop = nc.sync.nop()
            wait_clock.add_sem_waits(nop.ins, ScopedClock({None: pvc}))
    nc.sync.drain()
    nc._nrt_pseudo_barrier()
    assert self.sems is not None
    popped = nc._tile_sem_poison_stack.pop()
    assert popped is self._sem_poison
    nc.clear_and_free_semaphores(list(self.sems.allocated().values()))
    nc._nrt_pseudo_barrier()


tile.TileContext._drain_and_barrier = _patched_drain_and_barrier


def _split_multi_waits(nc):
    import bass_rust

    for f in nc.m.functions:
        for bb in f.blocks:
            out = []
            changed = False
            for ins in bb.instructions:
                si = ins.sync_info
                waits = list(si.on_wait) if si is not None else []
                if len(waits) > 1:
                    changed = True
                    for w in waits[:-1]:
                        nop = mybir.InstNoOp(
                            name=f"I-wsplit-{nc.next_id()}", ins=[], outs=[]
                        )
                        nop.engine = ins.engine
                        nop.sync_info = bass_rust.SyncInfo(on_wait=[w], on_update=[])
                        out.append(nop)
                    ins.sync_info = bass_rust.SyncInfo(
                        on_wait=[waits[-1]], on_update=list(si.on_update)
                    )
                out.append(ins)
            if changed:
                bb.instructions = out


# ---------------------------------------------------------------------------
# Kernel build
# ---------------------------------------------------------------------------

def build_nc(repeat=1):
    nc = bass.Bass(num_devices=M)
    x = nc.declare_dram_parameter("x", [B, D], F32, isOutput=False)
    out = nc.declare_dram_parameter("out", [BL, 24], F32, isOutput=True)
    for _rep in range(repeat):
        _build_body(nc, x, out)
    _split_multi_waits(nc)
    return nc


def _build_body(nc, x, out):
    with tile.TileContext(nc) as tc, ExitStack() as octx:
        cpool = octx.enter_context(tc.tile_pool(name="const", bufs=1))
        ident_bf = cpool.tile([128, 128], BF16)
        make_identity(nc, ident_bf[:])
        # identity * 1000 for the diagonal (self-similarity) mask
        i1000 = cpool.tile([128, 128], F32)
        nc.gpsimd.memset(i1000[:], 0.0)
        nc.gpsimd.affine_select(
            out=i1000[:], in_=i1000[:], compare_op=ALU.not_equal,
            fill=1000.0, base=0, pattern=[[-1, 128]], channel_multiplier=1,
        )
        # off[p, i] = 512 * (i // 8): column-block base for candidate i
        off_i = cpool.tile([128, 128], I32)
        nc.gpsimd.iota(off_i[:], pattern=[[512, 16], [0, 8]], base=0,
                       channel_multiplier=0)
        off_f = cpool.tile([128, 128], F32)
        nc.scalar.copy(off_f[:], off_i[:])

        # xT stored as 8 column-chunks of 1024 per d-tile, so phase-2 matmuls
        # can start as soon as the first chunks are normalized+transposed.
        xt_pool = octx.enter_context(tc.tile_pool(name="xt", bufs=1))
        xt = [
            [
                xt_pool.tile([128, 1024], BF16, tag=f"xt{d}_{ch}",
                             name=f"xt{d}_{ch}")
                for ch in range(8)
            ]
            for d in range(4)
        ]

        # All pools open simultaneously: phase 1 and phase 2 overlap, so the
        # PSUM pools must not share banks (2 transpose + 6 matmul = 8 banks).
        ld = octx.enter_context(tc.tile_pool(name="ld", bufs=3))
        sm = octx.enter_context(tc.tile_pool(name="sm", bufs=4))
        tpp = octx.enter_context(tc.tile_pool(name="tp", bufs=2, space="PSUM"))
        mm = octx.enter_context(tc.tile_pool(name="mm", bufs=6, space="PSUM"))
        cand = octx.enter_context(tc.tile_pool(name="cand", bufs=1))
        fin = octx.enter_context(tc.tile_pool(name="fin", bufs=2))

        # Phase 1: normalize rows, cast bf16, transpose into xT
        if True:
            for rt in range(B // 128):
                xtile = ld.tile([128, D], F32, tag="xtile")
                nc.sync.dma_start(xtile[:], x[ts(rt, 128), :])
                sq = ld.tile([128, D], F32, tag="sq")
                ss = sm.tile([128, 1], F32, tag="ss")
                nc.scalar.activation(sq[:], xtile[:], AF.Square,
                                     accum_out=ss[:])
                nrm = sm.tile([128, 1], F32, tag="nrm")
                nc.scalar.sqrt(nrm[:], ss[:])
                rcp = sm.tile([128, 1], F32, tag="rcp")
                nc.vector.reciprocal(rcp[:], nrm[:])
                xbf = ld.tile([128, D], BF16, tag="xbf")
                nc.scalar.activation(xbf[:], xtile[:], AF.Copy, scale=rcp[:])
                for d4 in range(4):
                    # HWDGE transpose on the ACT-issued queue (keeps the SP
                    # queues free of xbar-mode transitions)
                    nc.scalar.dma_start_transpose(
                        out=xt[d4][rt // 8][:, ts(rt % 8, 128)],
                        in_=xbf[:, ts(d4, 128)],
                    )

        # Phase 2: stripe matmul + per-block top-8. Column-groups of 4 blocks
        # outermost so the first matmuls only need the first 2 xT chunks.
        if True:
            vals = [
                cand.tile([128, 128], F32, tag=f"VALS{m}", name=f"VALS{m}")
                for m in range(NRT)
            ]
            idx = [
                cand.tile([128, 128], U32, tag=f"IDX{m}", name=f"IDX{m}")
                for m in range(NRT)
            ]
            for grp in range(4):
                cbs = list(range(grp * 4, grp * 4 + 4))
                for m in range(NRT):
                    pss = [
                        mm.tile([128, 512], F32, tag="ps", name=f"ps_{m}_{cb}")
                        for cb in cbs
                    ]
                    for d4 in range(4):
                        lhsT = xt[d4][0][:, ts(m, 128)]
                        for j, cb in enumerate(cbs):
                            nc.tensor.matmul(
                                pss[j][:], lhsT,
                                xt[d4][cb // 2][:, ts(cb % 2, 512)],
                                start=(d4 == 0), stop=(d4 == 3),
                            )
                    for j, cb in enumerate(cbs):
                        if cb == m // 4:
                            o = (m % 4) * 128
                            nc.vector.tensor_tensor(
                                pss[j][:, o:o + 128], pss[j][:, o:o + 128],
                                i1000[:], op=ALU.subtract,
                            )
                        nc.vector.max(vals[m][:, cb * 8:cb * 8 + 8], pss[j][:])
                        nc.vector.max_index(
                            idx[m][:, cb * 8:cb * 8 + 8],
                            vals[m][:, cb * 8:cb * 8 + 8], pss[j][:],
                        )
            # Phase 3: pack p = round(v * 2048) * 8192 + (idx + 512 * (i//8)),
            # then merge the 128 candidates to sorted top-24.
            for m in range(NRT):
                vq = fin.tile([128, 128], F32, tag="vq")
                nc.scalar.activation(vq[:], vals[m][:], AF.Copy, scale=2048.0,
                                     bias=MAGIC)
                q = fin.tile([128, 128], F32, tag="q")
                nc.vector.tensor_scalar_add(q[:], vq[:], -MAGIC)
                idxf = fin.tile([128, 128], F32, tag="idxf")
                nc.scalar.copy(idxf[:], idx[m][:])
                t1 = fin.tile([128, 128], F32, tag="t1")
                nc.vector.tensor_tensor(t1[:], idxf[:], off_f[:], op=ALU.add)
                p0 = fin.tile([128, 128], F32, tag="p0")
                nc.vector.scalar_tensor_tensor(
                    p0[:], in0=q[:], scalar=8192.0, in1=t1[:],
                    op0=ALU.mult, op1=ALU.add,
                )
                pv = fin.tile([128, 24], F32, tag="pv")
                p1 = fin.tile([128, 128], F32, tag="p1")
                p2 = fin.tile([128, 128], F32, tag="p2")
                nc.vector.max(pv[:, 0:8], p0[:])
                nc.vector.match_replace(p1[:], pv[:, 0:8], p0[:], NEG)
                nc.vector.max(pv[:, 8:16], p1[:])
                nc.vector.match_replace(p2[:], pv[:, 8:16], p1[:], NEG)
                nc.vector.max(pv[:, 16:24], p2[:])
                nc.sync.dma_start(out[ts(m, 128), :], pv[:])


_NC = None


def _get_nc():
    global _NC
    if _NC is None:
        _NC = build_nc()
    return _NC


def run_device(x32, trace=False, **kwargs):
    """Run the SPMD kernel; returns (pv [B, 24] f32, BassKernelResults)."""
    nc = _get_nc()
    in_maps = [
        {"x": np.ascontiguousarray(np.roll(x32, -c * BL, axis=0))}
        for c in range(M)
    ]
    res = run_bass_kernel_spmd(nc, in_maps, core_ids=list(range(M)),
                               trace=trace, **kwargs)
    pv = np.concatenate([res.results[c]["out"] for c in range(M)], axis=0)
    return pv, res


def decode_loss(pv, labels, k):
    """Decode packed top-24 -> (values, global column ids) -> BCE loss."""
    pv64 = pv.astype(np.float64)
    q = np.floor(pv64 / 8192.0)
    col = (pv64 - q * 8192.0).astype(np.int64)       # local column in [0, 8192)
    vhat = q / 2048.0                                 # quantized cosine
    vk = vhat[:, :k]
    ck = col[:, :k]
    core = np.arange(B) // BL                         # global row -> core
    gidx = (ck + (core * BL)[:, None]) % B            # local -> global column
    preds = (vk + 1.0) * 0.5
    t = (labels[gidx] == labels[:, None]).astype(np.float64)
    logp = np.maximum(np.log(preds), -100.0)
    log1mp = np.maximum(np.log1p(-preds), -100.0)
    loss = -(t * logp + (1.0 - t) * log1mp)
    return np.float32(loss.mean())


def kernel(batch, labels, k):
    k = int(k)
    assert 0 < k <= 24, f"kernel supports k <= 24, got {k}"
    x32 = np.asarray(batch, dtype=np.float32)
    assert x32.shape == (B, D)
    labels = np.asarray(labels)
    pv, _ = run_device(x32)
    return decode_loss(pv, labels, k)


# revision 37
# speedup vs baseline: 4943.2648x; 1.1893x over previous
"""Trainium2 Bass kernel for nn_BCE_Loss (retrieval_knn).

Distributed strategy (8 NeuronCores, SPMD):
  - Each core receives the full batch, ROTATED so that its own 1024 rows come
    first (row-stripe sharding with a replicated right operand; rotation makes
    the SPMD program identical across cores: core c's local row r == global row
    (r + 1024*c) % 8192, likewise columns).
  - On-device per core: L2-normalize rows (f32), cast bf16, transpose via PE
    into xT [512, 8192]; compute the [1024, 8192] cosine stripe tile-by-tile
    through PSUM (bf16 matmul, f32 accumulate); mask the self-match diagonal
    by subtracting 1000 on the (static, thanks to rotation) diagonal blocks;
    per 512-column block take top-8 values + indices (DVE max8/max_index);
    pack (round(v*2048), column) into a single f32 with exact integer
    arithmetic: p = round(v*2048)*8192 + col; merge the 128 candidates per row
    with 3 x (max8 + match_replace) -> sorted top-24 packed values.
  - Host: decode (value, column) from the packed top-24, map columns back to
    global ids, gather labels, and compute the BCE loss (tiny: 8192 x 20).

The per-block top-8 union provably contains the true top-k for k <= 8 per
block; for k=20 over 16 blocks the union miss probability is ~1e-6 per row
and empirically zero effect on the loss (validated vs the reference).
"""

from contextlib import ExitStack

import numpy as np

import concourse.bass as bass
import concourse.mybir as mybir
import concourse.tile as tile
from concourse.bass import ts
from concourse.bass_utils import run_bass_kernel_spmd
from concourse.masks import make_identity
from concourse.vector_clock import ScopedClock, VectorClock

F32 = mybir.dt.float32
BF16 = mybir.dt.bfloat16
U32 = mybir.dt.uint32
I32 = mybir.dt.int32
AF = mybir.ActivationFunctionType
ALU = mybir.AluOpType

B, D = 8192, 512
M = 8              # cores
BL = B // M        # 1024 rows per core
NRT = BL // 128    # 8 row tiles per core
NCB = B // 512     # 16 column blocks
MAGIC = 12582912.0  # 1.5 * 2**23: adding+subtracting rounds to nearest int
BIGMAGIC = 103079215104.0  # 1.5 * 2**36: rounds v*2^24 to multiples of 2^13
NEG = -3.0e38

# engine-assignment knobs (tuned via the timeline cost model)
USE_PACK = True      # pack col into value on DVE (else max_index path)
SCALE_ON_ACT = False  # normalize-scale op on ACT (else DVE)
XTCOPY_ACT_MOD = 2   # every Nth xT-copy goes to ACT (0 = all on DVE)


# ---------------------------------------------------------------------------
# Environment workarounds: this container's walrus accepts at most ONE sem
# wait per instruction, and its runtime crashes on the explicit EventSemaphore
# butterfly barrier TileContext emits at its tail.
# ---------------------------------------------------------------------------

def _patched_drain_and_barrier(self, tick_clock, wait_clock):
    nc = self.nc
    vc = tick_clock.global_clock
    n = len(vc)
    for p in range(n):
        t = vc[p]
        if t > 0:
            pvc = VectorClock([0] * n)
            pvc.require_at_least(p, t)
            nop = nc.sync.nop()
            wait_clock.add_sem_waits(nop.ins, ScopedClock({None: pvc}))
    nc.sync.drain()
    nc._nrt_pseudo_barrier()
    assert self.sems is not None
    popped = nc._tile_sem_poison_stack.pop()
    assert popped is self._sem_poison
    nc.clear_and_free_semaphores(list(self.sems.allocated().values()))
    nc._nrt_pseudo_barrier()


tile.TileContext._drain_and_barrier = _patched_drain_and_barrier


def _split_multi_waits(nc):
    import bass_rust

    for f in nc.m.functions:
        for bb in f.blocks:
            out = []
            changed = False
            for ins in bb.instructions:
                si = ins.sync_info
                waits = list(si.on_wait) if si is not None else []
                if len(waits) > 1:
                    changed = True
                    for w in waits[:-1]:
                        nop = mybir.InstNoOp(
                            name=f"I-wsplit-{nc.next_id()}", ins=[], outs=[]
                        )
                        nop.engine = ins.engine
                        nop.sync_info = bass_rust.SyncInfo(on_wait=[w], on_update=[])
                        out.append(nop)
                    ins.sync_info = bass_rust.SyncInfo(
                        on_wait=[waits[-1]], on_update=list(si.on_update)
                    )
                out.append(ins)
            if changed:
                bb.instructions = out


# ---------------------------------------------------------------------------
# Kernel build
# ---------------------------------------------------------------------------

def build_nc(repeat=1):
    nc = bass.Bass(num_devices=M)
    x = nc.declare_dram_parameter("x", [B, D], F32, isOutput=False)
    out = nc.declare_dram_parameter("out", [BL, 24], F32, isOutput=True)
    for _rep in range(repeat):
        _build_body(nc, x, out)
    _split_multi_waits(nc)
    return nc


def _build_body(nc, x, out):
    with tile.TileContext(nc) as tc, ExitStack() as octx:
        cpool = octx.enter_context(tc.tile_pool(name="const", bufs=1))
        ident_bf = cpool.tile([128, 128], BF16)
        make_identity(nc, ident_bf[:])
        # identity * 1000 for the diagonal (self-similarity) mask
        i1000 = cpool.tile([128, 128], F32)
        nc.gpsimd.memset(i1000[:], 0.0)
        nc.gpsimd.affine_select(
            out=i1000[:], in_=i1000[:], compare_op=ALU.not_equal,
            fill=1000.0, base=0, pattern=[[-1, 128]], channel_multiplier=1,
        )
        # off[p, i] = 1024 * (i // 8): scan-block base for candidate slot i
        off_i = cpool.tile([128, 64], I32)
        nc.gpsimd.iota(off_i[:], pattern=[[1024, 8], [0, 8]], base=0,
                       channel_multiplier=0)
        off_f = cpool.tile([128, 64], F32)
        nc.scalar.copy(off_f[:], off_i[:])
        # iota 0..1023 (local column within a scan block)
        iota_i = cpool.tile([128, 1024], I32)
        nc.gpsimd.iota(iota_i[:], pattern=[[1, 1024]], base=0,
                       channel_multiplier=0)
        iota_f = cpool.tile([128, 1024], F32)
        nc.scalar.copy(iota_f[:], iota_i[:])

        # xT stored as 8 column-chunks of [128, 4 d-tiles, 1024 cols], so
        # phase-2 matmuls can start as soon as the first chunks are ready and
        # each row-tile's 4 transposes land with a single DVE copy.
        xt_pool = octx.enter_context(tc.tile_pool(name="xt", bufs=1))
        xt = [
            xt_pool.tile([128, 4, 1024], BF16, tag=f"xt_{ch}", name=f"xt_{ch}")
            for ch in range(8)
        ]

        # All pools open simultaneously: phase 1 and phase 2 overlap, so the
        # PSUM pools must not share banks (2 transpose + 6 matmul = 8 banks).
        ld = octx.enter_context(tc.tile_pool(name="ld", bufs=3))
        sm = octx.enter_context(tc.tile_pool(name="sm", bufs=4))
        tpp = octx.enter_context(tc.tile_pool(name="tp", bufs=2, space="PSUM"))
        mm = octx.enter_context(tc.tile_pool(name="mm", bufs=3, space="PSUM"))
        sb = octx.enter_context(tc.tile_pool(name="sb", bufs=6))
        cand = octx.enter_context(tc.tile_pool(name="cand", bufs=1))
        fin = octx.enter_context(tc.tile_pool(name="fin", bufs=2))

        # Phase 1: normalize rows, cast bf16, transpose into xT
        if True:
            for rt in range(B // 128):
                xtile = ld.tile([128, D], F32, tag="xtile")
                nc.sync.dma_start(xtile[:], x[ts(rt, 128), :])
                sq = ld.tile([128, D], F32, tag="sq")
                ss = sm.tile([128, 1], F32, tag="ss")
                nc.scalar.activation(sq[:], xtile[:], AF.Square,
                                     accum_out=ss[:])
                nrm = sm.tile([128, 1], F32, tag="nrm")
                nc.scalar.sqrt(nrm[:], ss[:])
                rcp = sm.tile([128, 1], F32, tag="rcp")
                nc.vector.reciprocal(rcp[:], nrm[:])
                xbf = ld.tile([128, D], BF16, tag="xbf")
                if SCALE_ON_ACT:
                    nc.scalar.activation(xbf[:], xtile[:], AF.Copy,
                                         scale=rcp[:])
                else:
                    nc.vector.tensor_scalar_mul(xbf[:], xtile[:], rcp[:])
                tps = tpp.tile([128, 512], BF16, tag="tp")
                for d4 in range(4):
                    nc.tensor.transpose(tps[:, ts(d4, 128)], xbf[:, ts(d4, 128)],
                                        ident_bf[:])
                # one strided copy drops all 4 transposed blocks into the chunk
                ceng = (nc.scalar if XTCOPY_ACT_MOD and rt % XTCOPY_ACT_MOD == 0
                        else nc.vector)
                if ceng is nc.scalar:
                    nc.scalar.copy(
                        xt[rt // 8][:, :, ts(rt % 8, 128)],
                        tps[:].rearrange("p (d c) -> p d c", c=128),
                    )
                else:
                    nc.vector.tensor_copy(
                        xt[rt // 8][:, :, ts(rt % 8, 128)],
                        tps[:].rearrange("p (d c) -> p d c", c=128),
                    )

        # Phase 2: stripe matmul + top-8 per 1024-column scan block (8 blocks
        # per row-tile). Scan-block groups outermost so the first matmuls only
        # need the first xT chunks.
        if True:
            vals = [
                cand.tile([128, 64], F32, tag=f"VALS{m}", name=f"VALS{m}")
                for m in range(NRT)
            ]
            idx = [
                cand.tile([128, 64], U32, tag=f"IDX{m}", name=f"IDX{m}")
                for m in range(NRT)
            ] if not USE_PACK else None
            nblk = 0
            for grp in range(4):
                sbks = [grp * 2, grp * 2 + 1]  # 1024-col scan blocks
                for m in range(NRT):
                    pss = [
                        mm.tile([128, 1024], F32, tag="ps", name=f"ps_{m}_{b2}")
                        for b2 in sbks
                    ]
                    for d4 in range(4):
                        lhsT = xt[0][:, d4, ts(m, 128)]
                        for j, b2 in enumerate(sbks):
                            for h in range(2):
                                nc.tensor.matmul(
                                    pss[j][:, ts(h, 512)], lhsT,
                                    xt[b2][:, d4, ts(h, 512)],
                                    start=(d4 == 0), stop=(d4 == 3),
                                )
                    for j, b2 in enumerate(sbks):
                        if b2 == 0:
                            # all diagonals live in local columns m*128..+127
                            o = m * 128
                            nc.vector.tensor_tensor(
                                pss[j][:, o:o + 128], pss[j][:, o:o + 128],
                                i1000[:], op=ALU.subtract,
                            )
                        if USE_PACK:
                            # Evacuate PSUM on ACT, fusing the rounding:
                            #   t = v*2^24 + 1.5*2^36 (rounds to the 2^13 grid)
                            sbt = sb.tile([128, 1024], F32, tag="sb")
                            nc.scalar.activation(sbt[:], pss[j][:], AF.Copy,
                                                 scale=16777216.0, bias=BIGMAGIC)
                            # pack local column: pl = (t - BIG) + iota
                            pkt = sb.tile([128, 1024], F32, tag="pk")
                            nc.vector.scalar_tensor_tensor(
                                pkt[:], in0=sbt[:], scalar=BIGMAGIC,
                                in1=iota_f[:], op0=ALU.subtract, op1=ALU.add,
                            )
                            nblk += 1
                            nc.vector.max(vals[m][:, b2 * 8:b2 * 8 + 8], pkt[:])
                        else:
                            sbt = sb.tile([128, 1024], F32, tag="sb")
                            nc.scalar.copy(sbt[:], pss[j][:])
                            nc.vector.max(vals[m][:, b2 * 8:b2 * 8 + 8], sbt[:])
                            nc.vector.max_index(
                                idx[m][:, b2 * 8:b2 * 8 + 8],
                                vals[m][:, b2 * 8:b2 * 8 + 8], sbt[:],
                            )
            # Phase 3: add the scan-block base into the column field, then
            # merge the 64 packed candidates to sorted top-24.
            for m in range(NRT):
                p0 = fin.tile([128, 64], F32, tag="p0")
                if USE_PACK:
                    nc.vector.tensor_tensor(p0[:], vals[m][:], off_f[:],
                                            op=ALU.add)
                else:
                    vq = fin.tile([128, 64], F32, tag="vq")
                    nc.scalar.activation(vq[:], vals[m][:], AF.Copy,
                                         scale=2048.0, bias=MAGIC)
                    q = fin.tile([128, 64], F32, tag="q")
                    nc.vector.tensor_scalar_add(q[:], vq[:], -MAGIC)
                    idxf = fin.tile([128, 64], F32, tag="idxf")
                    nc.scalar.copy(idxf[:], idx[m][:])
                    t1 = fin.tile([128, 64], F32, tag="t1")
                    nc.vector.tensor_tensor(t1[:], idxf[:], off_f[:],
                                            op=ALU.add)
                    nc.vector.scalar_tensor_tensor(
                        p0[:], in0=q[:], scalar=8192.0, in1=t1[:],
                        op0=ALU.mult, op1=ALU.add,
                    )
                pv = fin.tile([128, 24], F32, tag="pv")
                p1 = fin.tile([128, 64], F32, tag="p1")
                p2 = fin.tile([128, 64], F32, tag="p2")
                nc.vector.max(pv[:, 0:8], p0[:])
                nc.vector.match_replace(p1[:], pv[:, 0:8], p0[:], NEG)
                nc.vector.max(pv[:, 8:16], p1[:])
                nc.vector.match_replace(p2[:], pv[:, 8:16], p1[:], NEG)
                nc.vector.max(pv[:, 16:24], p2[:])
                nc.sync.dma_start(out[ts(m, 128), :], pv[:])


_NC = None


def _get_nc():
    global _NC
    if _NC is None:
        _NC = build_nc()
    return _NC


def run_device(x32, trace=False, **kwargs):
    """Run the SPMD kernel; returns (pv [B, 24] f32, BassKernelResults)."""
    nc = _get_nc()
    in_maps = [
        {"x": np.ascontiguousarray(np.roll(x32, -c * BL, axis=0))}
        for c in range(M)
    ]
    res = run_bass_kernel_spmd(nc, in_maps, core_ids=list(range(M)),
                               trace=trace, **kwargs)
    pv = np.concatenate([res.results[c]["out"] for c in range(M)], axis=0)
    return pv, res


def decode_loss(pv, labels, k):
    """Decode packed top-24 -> (values, global column ids) -> BCE loss."""
    pv64 = pv.astype(np.float64)
    q = np.floor(pv64 / 8192.0)
    col = (pv64 - q * 8192.0).astype(np.int64)       # local column in [0, 8192)
    vhat = q / 2048.0                                 # quantized cosine
    vk = vhat[:, :k]
    ck = col[:, :k]
    core = np.arange(B) // BL                         # global row -> core
    gidx = (ck + (core * BL)[:, None]) % B            # local -> global column
    preds = (vk + 1.0) * 0.5
    t = (labels[gidx] == labels[:, None]).astype(np.float64)
    logp = np.maximum(np.log(preds), -100.0)
    log1mp = np.maximum(np.log1p(-preds), -100.0)
    loss = -(t * logp + (1.0 - t) * log1mp)
    return np.float32(loss.mean())


def kernel(batch, labels, k):
    k = int(k)
    assert 0 < k <= 24, f"kernel supports k <= 24, got {k}"
    x32 = np.asarray(batch, dtype=np.float32)
    assert x32.shape == (B, D)
    labels = np.asarray(labels)
    pv, _ = run_device(x32)
    return decode_loss(pv, labels, k)
